# revision 13
# baseline (speedup 1.0000x reference)
"""Trainium2 Bass kernel for nn_EquivariantModel (e3nn-style equivariant net).

Architecture (per batch row): two blocks of
  {o3.Linear x2 -> FullyConnectedTensorProduct('Mx0e+Mx1o' ^2 -> 128x0e+128x1o)
   -> learnable tanh gate -> o3.Linear}, then a final o3.Linear.

Strategy: data-parallel over batch (8 cores x 1024 rows), feature-major
activations [feature, batch] on-device.  The tensor product is computed as
z[(u,v), b] = f1[u,b] * f2[v,b] (f16, formed on DVE with PE-assisted
partition broadcast of the f1 factor), followed by PSUM-accumulated
matmuls z^T @ W[(u,v), w] over k-tiles.  Linears/gates run in fp32.
All normalization constants are folded into the weights host-side.
"""

import sys
import numpy as np
import ml_dtypes

if '/opt/trn_rl_repo' not in sys.path:
    sys.path.insert(0, '/opt/trn_rl_repo')

B, M_IN, M_HID = 8192, 64, 128
N_CORES = 8
BC = B // N_CORES            # batch per core
CH = 512                     # chunk of batch processed per matmul group
NCH = BC // CH
TANH_GAIN = 1.5927116870880127

F32 = None  # set after mybir import
BF16 = None

_CACHE = {}


def _build_program():
    import concourse.mybir as mybir
    import concourse.tile as tile
    from concourse import bacc
    from contextlib import ExitStack

    f32 = mybir.dt.float32
    f16 = mybir.dt.float16

    nc = bacc.Bacc("TRN2", target_bir_lowering=False)

    # ---- DRAM I/O ----
    s0 = nc.dram_tensor("s0", [64, BC], f32, kind="ExternalInput")
    v0 = nc.dram_tensor("v0", [192, BC], f32, kind="ExternalInput")  # rows i*64+u

    dram = {}
    for blk, M in (("b1", 64), ("b2", 128)):
        for nm in ("l1_w0", "l1_w1", "l2_w0", "l2_w1"):
            dram[f"{blk}_{nm}"] = nc.dram_tensor(f"{blk}_{nm}", [M, M], f32,
                                                 kind="ExternalInput")
        K = M * M
        for nm in ("ss", "vv", "sv", "vs"):
            # p-major layout: [128, K//128, 128]
            dram[f"{blk}_w_{nm}"] = nc.dram_tensor(
                f"{blk}_w_{nm}", [128, K // 128, 128], f16, kind="ExternalInput")
        for nm in ("g_ws", "g_wg", "g_wv", "o_w0", "o_w1"):
            dram[f"{blk}_{nm}"] = nc.dram_tensor(f"{blk}_{nm}", [128, 128], f32,
                                                 kind="ExternalInput")
    dram["f_w0"] = nc.dram_tensor("f_w0", [128, 64], f32, kind="ExternalInput")
    dram["f_w1"] = nc.dram_tensor("f_w1", [128, 64], f32, kind="ExternalInput")
    idm = nc.dram_tensor("idm", [128, 128], f16, kind="ExternalInput")
    di64 = nc.dram_tensor("di64", [64, 4096], f16, kind="ExternalInput")

    out_d = nc.dram_tensor("out", [256, BC], f32, kind="ExternalOutput")

    with ExitStack() as ctx:
        tc = ctx.enter_context(tile.TileContext(nc))
        consts = ctx.enter_context(tc.tile_pool(name="consts", bufs=1))
        acts = ctx.enter_context(tc.tile_pool(name="acts", bufs=1))
        wstream = ctx.enter_context(tc.tile_pool(name="wstream", bufs=2))
        bc_ps = ctx.enter_context(tc.tile_pool(name="bc_ps", bufs=4, space="PSUM"))
        acc_ps = ctx.enter_context(tc.tile_pool(name="acc_ps", bufs=1, space="PSUM"))
        bc_sb = ctx.enter_context(tc.tile_pool(name="bc_sb", bufs=2))
        z_pool = ctx.enter_context(tc.tile_pool(name="z", bufs=6))
        tmp = ctx.enter_context(tc.tile_pool(name="tmp", bufs=3))

        # ---- load constants ----
        W = {}
        for name, t in dram.items():
            if name.startswith("b1_w_"):
                w = consts.tile([128, 32, 128], f16, tag=name)
                nc.sync.dma_start(w[:], t[:])
                W[name] = w
            elif name.startswith("b2_w_"):
                W[name] = t  # streamed
            else:
                shp = list(t.shape)
                w = consts.tile(shp, f32, tag=name)
                nc.sync.dma_start(w[:], t[:])
                W[name] = w
        e1_sb = consts.tile([1, 128], f16, tag="e1")
        nc.sync.dma_start(e1_sb[:], e1[:])
        e2_sb = consts.tile([2, 128], f16, tag="e2")
        nc.sync.dma_start(e2_sb[:], e2[:])

        # ---- input activations ----
        sT = acts.tile([64, BC], f32, tag="in_s")
        nc.sync.dma_start(sT[:], s0[:])
        vT = []
        for i in range(3):
            t = acts.tile([64, BC], f32, tag=f"in_v{i}")
            nc.sync.dma_start(t[:], v0[i * 64:(i + 1) * 64, :])
            vT.append(t)

        def linear(w_sb, x_sb, Min, Mout, out_sb, out_rows=None, second_rows=None):
            """out = w^T x, feature-major; optional duplicate write to rows."""
            for c in range(NCH):
                sl = slice(c * CH, (c + 1) * CH)
                ps = bc_ps.tile([128, CH], f32, tag="bc")
                nc.tensor.matmul(ps[:Mout], w_sb[:Min, :Mout], x_sb[:Min, sl],
                                 start=True, stop=True)
                r0 = out_rows or slice(0, Mout)
                nc.scalar.copy(out_sb[r0, sl], ps[:Mout])
                if second_rows is not None:
                    nc.scalar.copy(out_sb[second_rows, sl], ps[:Mout])

        def block(blk, U, s_in, v_in, s_out, v_out):
            """One equivariant block. s_in [U, BC] f32, v_in [3][U, BC] f32.
            Writes s_out [128, BC] f32, v_out [3][128, BC] f32."""
            V = U
            K = U * V
            KT = K // 128
            g = 128 // V

            # --- l1 / l2 linears -> bf16 operands ---
            # bcast-side factors (f1): s1b [U, BC], v1b[i] [U, BC]
            s1b = acts.tile([U, BC], f16, tag="s1b")
            v1b = [acts.tile([U, BC], f16, tag=f"v1b{i}") for i in range(3)]
            # tile-side factors (f2), partition-replicated to 128 rows
            s2r = acts.tile([128, BC], f16, tag="s2r")
            v2r = [acts.tile([128, BC], f16, tag=f"v2r{i}") for i in range(3)]

            dup = slice(64, 128) if g == 2 else None
            linear(W[f"{blk}_l1_w0"], s_in, U, U, s1b)
            for i in range(3):
                linear(W[f"{blk}_l1_w1"], v_in[i], U, U, v1b[i])
            linear(W[f"{blk}_l2_w0"], s_in, U, U, s2r, second_rows=dup)
            for i in range(3):
                linear(W[f"{blk}_l2_w1"], v_in[i], U, U, v2r[i], second_rows=dup)

            # --- tensor product ---
            tp_s = acts.tile([128, BC], f32, tag="tp_s")
            tp_v = [acts.tile([128, BC], f32, tag=f"tp_v{i}") for i in range(3)]

            for c in range(NCH):
                sl = slice(c * CH, (c + 1) * CH)
                acc_s = acc_ps.tile([128, CH], f32, tag="acc_s")
                acc_v = [acc_ps.tile([128, CH], f32, tag=f"acc_v{i}")
                         for i in range(3)]
                for kt in range(KT):
                    u0 = kt * g
                    if blk == "b1":
                        wss = W["b1_w_ss"][:, kt]
                        wvv = W["b1_w_vv"][:, kt]
                        wsv = W["b1_w_sv"][:, kt]
                        wvs = W["b1_w_vs"][:, kt]
                    else:
                        wss = wstream.tile([128, 128], f16, tag="wss")
                        nc.sync.dma_start(wss[:], W["b2_w_ss"][:, kt])
                        wvv = wstream.tile([128, 128], f16, tag="wvv")
                        nc.sync.dma_start(wvv[:], W["b2_w_vv"][:, kt])
                        wsv = wstream.tile([128, 128], f16, tag="wsv")
                        nc.sync.dma_start(wsv[:], W["b2_w_sv"][:, kt])
                        wvs = wstream.tile([128, 128], f16, tag="wvs")
                        nc.sync.dma_start(wvs[:], W["b2_w_vs"][:, kt])

                    # partition-broadcast of f1 rows via selector matmul
                    bps = bc_ps.tile([128, CH], f32, tag="bc")
                    nc.tensor.matmul(bps, E[:g], s1b[u0:u0 + g, sl],
                                     start=True, stop=True)
                    bs = bc_sb.tile([128, CH], f16, tag="bcs")
                    nc.scalar.copy(bs, bps)
                    bv = []
                    for i in range(3):
                        p = bc_ps.tile([128, CH], f32, tag="bc")
                        nc.tensor.matmul(p, E[:g], v1b[i][u0:u0 + g, sl],
                                         start=True, stop=True)
                        t = bc_sb.tile([128, CH], f16, tag=f"bcv{i}")
                        nc.scalar.copy(t, p)
                        bv.append(t)

                    first = kt == 0
                    last = kt == KT - 1
                    # scalar output: ss + vv paths accumulate into acc_s
                    z = z_pool.tile([128, CH], f16, tag="z")
                    nc.vector.tensor_mul(z, bs, s2r[:, sl])
                    nc.tensor.matmul(acc_s, wss, z, start=first, stop=False)
                    for i in range(3):
                        z = z_pool.tile([128, CH], f16, tag="z")
                        nc.vector.tensor_mul(z, bv[i], v2r[i][:, sl])
                        nc.tensor.matmul(acc_s, wvv, z, start=False,
                                         stop=(last and i == 2))
                    # vector outputs: sv + vs paths
                    for i in range(3):
                        z = z_pool.tile([128, CH], f16, tag="z")
                        nc.vector.tensor_mul(z, bs, v2r[i][:, sl])
                        nc.tensor.matmul(acc_v[i], wsv, z, start=first, stop=False)
                    for i in range(3):
                        z = z_pool.tile([128, CH], f16, tag="z")
                        nc.vector.tensor_mul(z, bv[i], s2r[:, sl])
                        nc.tensor.matmul(acc_v[i], wvs, z, start=False, stop=last)

                nc.vector.tensor_copy(tp_s[:, sl], acc_s)
                for i in range(3):
                    nc.vector.tensor_copy(tp_v[i][:, sl], acc_v[i])

            # --- gate ---
            tanh_s = acts.tile([128, BC], f32, tag="tanh_s")
            gated_v = [acts.tile([128, BC], f32, tag=f"gated_v{i}")
                       for i in range(3)]
            for c in range(NCH):
                sl = slice(c * CH, (c + 1) * CH)
                ps = bc_ps.tile([128, CH], f32, tag="bc")
                nc.tensor.matmul(ps, W[f"{blk}_g_ws"], tp_s[:, sl],
                                 start=True, stop=True)
                nc.scalar.activation(tanh_s[:, sl], ps,
                                     mybir.ActivationFunctionType.Tanh)
                psg = bc_ps.tile([128, CH], f32, tag="bc")
                nc.tensor.matmul(psg, W[f"{blk}_g_wg"], tp_s[:, sl],
                                 start=True, stop=True)
                tg = tmp.tile([128, CH], f32, tag="tanh_g")
                nc.scalar.activation(tg, psg,
                                     mybir.ActivationFunctionType.Tanh)
                for i in range(3):
                    psv = bc_ps.tile([128, CH], f32, tag="bc")
                    nc.tensor.matmul(psv, W[f"{blk}_g_wv"], tp_v[i][:, sl],
                                     start=True, stop=True)
                    nc.vector.tensor_mul(gated_v[i][:, sl], psv, tg)

            # --- out linear ---
            linear(W[f"{blk}_o_w0"], tanh_s, 128, 128, s_out)
            for i in range(3):
                linear(W[f"{blk}_o_w1"], gated_v[i], 128, 128, v_out[i])

        # ---- block 1 ----
        s_b1 = acts.tile([128, BC], f32, tag="s_mid")
        v_b1 = [acts.tile([128, BC], f32, tag=f"v_mid{i}") for i in range(3)]
        block("b1", 64, sT, vT, s_b1, v_b1)

        # ---- block 2 ----
        s_b2 = acts.tile([128, BC], f32, tag="s_mid2")
        v_b2 = [acts.tile([128, BC], f32, tag=f"v_mid2{i}") for i in range(3)]
        block("b2", 128, s_b1, v_b1, s_b2, v_b2)

        # ---- final linear -> output ----
        fo = acts.tile([256, BC], f32, tag="final")  # rows: s 0:64, v_i 64+64i
        linear(W["f_w0"], s_b2, 128, 64, fo, out_rows=slice(0, 64))
        for i in range(3):
            linear(W["f_w1"], v_b2[i], 128, 64, fo,
                   out_rows=slice(64 + 64 * i, 128 + 64 * i))
        nc.sync.dma_start(out_d[:], fo[:])

    nc.finalize()
    return nc


def _host_prep(inputs):
    """Fold norm constants into weights; reorder/cast TP weights."""
    hf = np.float16
    d = {}
    for blk, M in (("b1", 64), ("b2", 128)):
        c_lin = np.float32(1.0 / np.sqrt(M))
        for nm in ("l1_w0", "l1_w1", "l2_w0", "l2_w1"):
            d[f"{blk}_{nm}"] = np.ascontiguousarray(
                inputs[f"{blk}_{nm}"] * c_lin, dtype=np.float32)
        c_tp = 1.0 / (M * np.sqrt(2.0))
        for nm, c in (("ss", c_tp), ("vv", c_tp / np.sqrt(3.0)),
                      ("sv", c_tp), ("vs", c_tp)):
            W = (inputs[f"{blk}_tp_{nm}"] * np.float32(c)).reshape(M * M, 128)
            # p-major: [128, K//128, 128]
            Wp = W.reshape(M * M // 128, 128, 128).transpose(1, 0, 2)
            d[f"{blk}_w_{nm}"] = np.ascontiguousarray(Wp).astype(hf)
        c_g = np.float32(1.0 / np.sqrt(128))
        for nm in ("g_ws", "g_wg", "g_wv"):
            d[f"{blk}_{nm}"] = np.ascontiguousarray(
                inputs[f"{blk}_{nm}"] * c_g, dtype=np.float32)
        c_og = np.float32(TANH_GAIN / np.sqrt(128))
        d[f"{blk}_o_w0"] = np.ascontiguousarray(
            inputs[f"{blk}_o_w0"] * c_og, dtype=np.float32)
        d[f"{blk}_o_w1"] = np.ascontiguousarray(
            inputs[f"{blk}_o_w1"] * c_og, dtype=np.float32)
    c_o = np.float32(1.0 / np.sqrt(128))
    d["f_w0"] = np.ascontiguousarray(inputs["f_w0"] * c_o, dtype=np.float32)
    d["f_w1"] = np.ascontiguousarray(inputs["f_w1"] * c_o, dtype=np.float32)
    d["idm"] = np.eye(128, dtype=np.float32).astype(hf)
    di = np.zeros((64, 4096), dtype=np.float32)
    for u in range(64):
        di[u, 64 * u:64 * u + 64] = 1.0
    d["di64"] = di.astype(hf)
    return d


def kernel(**inputs):
    from concourse.bass_utils import run_bass_kernel_spmd

    x = np.asarray(inputs["x"], dtype=np.float32)
    w = _host_prep({k: np.asarray(v, dtype=np.float32)
                    for k, v in inputs.items() if k != "x"})

    if "nc" not in _CACHE:
        _CACHE["nc"] = _build_program()
    nc = _CACHE["nc"]

    # shard + transpose to feature-major
    sT_full = np.ascontiguousarray(x[:, :64].T)                  # [64, B]
    v_full = x[:, 64:].reshape(B, 64, 3)
    vT_full = np.ascontiguousarray(v_full.transpose(2, 1, 0))    # [3, 64, B]
    in_maps = []
    for c in range(N_CORES):
        bs = slice(c * BC, (c + 1) * BC)
        m = dict(w)
        m["s0"] = np.ascontiguousarray(sT_full[:, bs])
        m["v0"] = np.ascontiguousarray(vT_full[:, :, bs]).reshape(192, BC)
        in_maps.append(m)

    res = run_bass_kernel_spmd(nc, in_maps, core_ids=list(range(N_CORES)))

    out = np.empty((B, 256), dtype=np.float32)
    for c in range(N_CORES):
        o = res.results[c]["out"]                                # [256, BC]
        bs = slice(c * BC, (c + 1) * BC)
        out[bs, :64] = o[:64].T
        # rows 64+64i+u = v comp i; ref layout col 64 + u*3 + i
        v = o[64:].reshape(3, 64, BC)
        out[bs, 64:] = v.transpose(2, 1, 0).reshape(BC, 192)
    return out
